# revision 1
# baseline (speedup 1.0000x reference)
"""GBST Trainium2 kernel (nn_GBST_42434276884940).

Self-contained: takes FULL inputs, shards batch over 8 NeuronCores
(2 rows/core), runs a Bass/Tile kernel per core, gathers full output.

Device algorithm per core (6144 positions = 48 chunks of 128):
- Indirect-DMA gather (one [128,1]-index call per chunk) from a
  host-built augmented table
  T4pad[v + 256*phi] = [table[v]+pe[phi] (256), (table[v]+pe[phi])@w (1), pad]
  -> X [128, 48, 320] (position g at partition g%128, chunk g//128).
  Fuses embedding lookup + positional encoding + score projection into
  one memory op.
- Scores: pooled per-position scores for block sizes 1..4 via small PE
  matmuls with constant block-pooling matrices (phase classes handle
  blocksize-3 misalignment); softmax + the tiny self-attention
  calibration as elementwise DVE/ACT ops on [128,12,4] tiles -> combine
  weights c4.
- The whole pool->repeat->weighted-combine->downsample(2) stack is one
  banded linear operator G (band ~[2t-3, 2t+4]). Its 128x128 chunk tiles
  (lhsT "Gsb") are built on the PE from C[l, j] = c4 * (j==p//2) masks,
  then applied: out2[t, h] = sum_k Gsb[k, t] * X[k, h].
  All matmuls are exact fp32 (fp32r measured ~2e-4 rel err = TF32-class;
  multi-index indirect DMA and the custom dma_gather ucode both broken
  on this runtime). HW rel err vs the jax reference: 4.5e-7.
"""

import os
import sys

import numpy as np

if "/opt/trn_rl_repo" not in sys.path:
    sys.path.insert(0, "/opt/trn_rl_repo")

import concourse.bass as bass
import concourse.tile as tile
from concourse import bacc, library_config, mybir
from concourse.bass_utils import run_bass_kernel_spmd

F32 = mybir.dt.float32
F32R = mybir.dt.float32r
I16 = mybir.dt.int16
I32 = mybir.dt.int32

MAX_BLOCK = 4
EMBED = 256
VOCAB = 256
BATCH = 16
SEQ = 3072
NCORES = 8
BLOC = BATCH // NCORES           # 2
NPOS = BLOC * SEQ                # 6144
NCHUNK = NPOS // 128             # 48
NGROUP = 4
GSZ = NCHUNK // NGROUP           # 12
ELEM = 320

SLOTS = [0, 3, 6, 9, 2, 5, 8, 11, 1, 4, 7, 10]   # slot s -> tau_l
SLOT_OF = {t: s for s, t in enumerate(SLOTS)}
CLASS_TAUL = [[0, 3, 6, 9], [2, 5, 8, 11], [1, 4, 7, 10]]


# ---------------------------------------------------------------- host consts

def _sinusoidal_pe(max_len, d):
    pos = np.arange(max_len, dtype=np.float32)[:, None]
    div = np.exp(np.arange(0, d, 2, dtype=np.float32) * (-np.log(10000.0) / d))
    pe = np.zeros((max_len, d), dtype=np.float32)
    pe[:, 0::2] = np.sin(pos * div)
    pe[:, 1::2] = np.cos(pos * div)
    return pe


def build_t4pad(embed_table, w_score):
    table = np.asarray(embed_table, dtype=np.float32)
    w = np.asarray(w_score, dtype=np.float32).reshape(EMBED)
    pe = _sinusoidal_pe(MAX_BLOCK, EMBED)
    t4 = np.zeros((MAX_BLOCK * VOCAB, ELEM), dtype=np.float32)
    for phi in range(MAX_BLOCK):
        rows = table + pe[phi][None, :]
        t4[phi * VOCAB:(phi + 1) * VOCAB, :EMBED] = rows
        t4[phi * VOCAB:(phi + 1) * VOCAB, EMBED] = rows @ w
    return t4


def phi_of_taul(tau_l):
    return (2 * tau_l) % 3


def build_smats():
    k = np.arange(128)
    mats = np.zeros((12, 128, 128), dtype=np.float32)
    mats[0] = 0.5 * np.eye(128, dtype=np.float32)
    mats[1] = 0.25 * (k[:, None] // 2 == k[None, :] // 2)
    mats[2] = 0.125 * (k[:, None] // 4 == k[None, :] // 4)
    for phi in range(3):
        mats[3 + phi] = (1 / 6) * ((k[:, None] + phi) // 3 == (k[None, :] + phi) // 3)
        mats[6 + phi] = (1 / 6) * ((128 + k[:, None] + phi) // 3 == (k[None, :] + phi) // 3)
        mats[9 + phi] = (1 / 6) * ((k[:, None] - 128 + phi) // 3 == (k[None, :] + phi) // 3)
    return mats


def build_m2mask():
    p = np.arange(128)
    j = np.arange(64)
    return (j[None, :] == p[:, None] // 2).astype(np.float32)


def build_idx_streams(input_ids):
    """Per-core int32 [128, 48]: idx[p, c] = augmented-table row for
    position g = 128*c + p (indirect-DMA gather: 320 elems per index)."""
    ids = np.asarray(input_ids).astype(np.int64)
    out = []
    g = np.arange(NPOS)
    row, l = g // SEQ, g % SEQ
    for core in range(NCORES):
        core_ids = ids[core * BLOC:(core + 1) * BLOC]
        vid = (core_ids[row, l] + 256 * (l % 4)).astype(np.int32)
        out.append(vid.reshape(NCHUNK, 128).T.copy())
    return out


# ---------------------------------------------------------------- device prog

def _r(ap):
    return ap


def emit_program(nc, nrep=1):
    t4pad_d = nc.dram_tensor("t4pad", [MAX_BLOCK * VOCAB, ELEM], F32,
                             kind="ExternalInput")
    idxs_d = nc.dram_tensor("idxs", [128, NCHUNK], I32,
                            kind="ExternalInput")
    smats_d = nc.dram_tensor("smats", [12, 128, 128], F32, kind="ExternalInput")
    m2_d = nc.dram_tensor("m2m", [128, 64], F32, kind="ExternalInput")
    out_d = nc.dram_tensor("out", [BLOC * SEQ // 2, EMBED], F32,
                           kind="ExternalOutput")

    with tile.TileContext(nc) as tc:
        with (
            tc.tile_pool(name="consts", bufs=1) as consts,
            tc.tile_pool(name="big", bufs=1) as big,
            tc.tile_pool(name="sm", bufs=2) as sm,
            tc.tile_pool(name="outsb", bufs=4) as outsb_pool,
            tc.tile_pool(name="scT_ps", bufs=1, space="PSUM") as scT_ps,
            tc.tile_pool(name="gmid_ps", bufs=1, space="PSUM") as gmid_ps,
            tc.tile_pool(name="glow_ps", bufs=1, space="PSUM") as glow_ps,
            tc.tile_pool(name="ghigh_ps", bufs=1, space="PSUM") as ghigh_ps,
            tc.tile_pool(name="out2_ps", bufs=2, space="PSUM") as out2_ps,
        ):
            # ---- constants to SBUF ----
            smats_sb = consts.tile([128, 12, 128], F32, tag="smats")
            nc.sync.dma_start(
                smats_sb[:],
                bass.AP(tensor=smats_d, offset=0,
                        ap=[[128, 128], [128 * 128, 12], [1, 128]]))
            m2_sb = consts.tile([128, 64], F32, tag="m2")
            nc.sync.dma_start(m2_sb[:], m2_d.ap()[:, :])
            idxs_sb = consts.tile([128, NCHUNK], I32, tag="ix")
            nc.sync.dma_start(idxs_sb[:], idxs_d.ap()[:, :])

            # ---- persistent big tensors ----
            X = big.tile([128, NCHUNK, ELEM], F32, tag="X")
            C = big.tile([128, NCHUNK, 4, 64], F32, tag="C")
            Gsb = big.tile([128, NCHUNK, 128], F32, tag="Gsb")
            d2 = big.tile([128, NCHUNK], F32, tag="d2")   # slot-ordered
            c4 = big.tile([128, NCHUNK, 4], F32, tag="c4")  # slot-ordered

            def mmat(out_ap, mi, rhs_ap, start, stop, f32r=False):
                # exact fp32 everywhere: fp32r measured at ~2e-4 rel err
                # (TF32-class), too coarse for this problem.
                lhsT = smats_sb[:, mi, :].bitcast(F32)
                rhs = rhs_ap.bitcast(F32)
                nc.tensor.matmul(out=out_ap, lhsT=lhsT, rhs=rhs,
                                 start=start, stop=stop,
                                 skip_group_check=True)

            def emit_gather(g):
                g0 = g * GSZ
                # multi-index indirect DMA is broken on HW: one call per
                # chunk ([128,1] indices -> 128 rows x 1280 B)
                for c in range(GSZ):
                    nc.gpsimd.indirect_dma_start(
                        out=X[:, g0 + c, :], out_offset=None,
                        in_=t4pad_d.ap()[:, :],
                        in_offset=bass.IndirectOffsetOnAxis(
                            ap=idxs_sb[:, g0 + c:g0 + c + 1], axis=0))
                # d2 = 2*d (score path feeds half-scaled matrices),
                # written in slot order: class c block <- tau_l stride-3 run
                for c in range(3):
                    t0 = CLASS_TAUL[c][0]
                    nc.scalar.mul(
                        d2[:, g0 + 4 * c:g0 + 4 * c + 4].unsqueeze(2),
                        X[:, g0 + t0:g0 + t0 + 10:3, EMBED:EMBED + 1], 2.0)

            def emit_scores(g):
                g0 = g * GSZ
                # m-major psum layout [128, 4 m, 12 slot]; every MM writes a
                # contiguous slot run (slot space makes up/dn sources
                # contiguous class blocks too).
                scT = scT_ps.tile([128, 4, GSZ], F32, tag="scT")
                mmat(scT[:, 0, :], 0, d2[:, g0:g0 + GSZ], True, False, False)
                mmat(scT[:, 1, :], 1, d2[:, g0:g0 + GSZ], False, False, False)
                mmat(scT[:, 3, :], 2, d2[:, g0:g0 + GSZ], False, False, False)
                # m=3 diag: slot block 4c:4c+4 <- same slots
                for c in range(3):
                    phi = phi_of_taul(CLASS_TAUL[c][0])
                    mmat(scT[:, 2, 4 * c:4 * c + 4], 3 + phi,
                         d2[:, g0 + 4 * c:g0 + 4 * c + 4], False, False, False)
                # up: (class c out slots, source slots); dn likewise
                up_sc = [(0, 0, 4, 8), (1, 4, 3, 1), (2, 8, 4, 4)]
                dn_sc = [(0, 1, 3, 4), (1, 4, 4, 8), (2, 8, 4, 0)]
                for plan, base in ((up_sc, 6), (dn_sc, 9)):
                    for c, o0, on, s0 in plan:
                        phi = phi_of_taul(CLASS_TAUL[c][0])
                        mmat(scT[:, 2, o0:o0 + on], base + phi,
                             d2[:, g0 + s0:g0 + s0 + on], False, False, False)
                if g % 2 == 0:   # up-fix: slot 7 (tau_l 11) <- next grp slot 0
                    mmat(scT[:, 2, 7:8], 6 + phi_of_taul(11),
                         d2[:, (g + 1) * GSZ:(g + 1) * GSZ + 1], False, False,
                         False)
                else:            # dn-fix: slot 0 <- prev group slot 7
                    mmat(scT[:, 2, 0:1], 9 + phi_of_taul(0),
                         d2[:, g0 - GSZ + 7:g0 - GSZ + 8], False, True, False)

                # softmax + calibration (scS transposed to [128, slot, m])
                scS = sm.tile([128, GSZ, 4], F32, tag="scS")
                base_ap = scT[:]
                scT_t = bass.AP(tensor=base_ap.tensor, offset=base_ap.offset,
                                ap=[list(base_ap.ap[0]), list(base_ap.ap[2]),
                                    list(base_ap.ap[1])])
                nc.vector.tensor_copy(out=scS[:], in_=scT_t)
                ex = sm.tile([128, GSZ, 4], F32, tag="ex")
                nc.scalar.activation(out=ex[:], in_=scS[:],
                                     func=mybir.ActivationFunctionType.Exp)
                Z = sm.tile([128, GSZ], F32, tag="Z")
                nc.vector.tensor_reduce(out=Z[:], in_=ex[:],
                                        axis=mybir.AxisListType.X,
                                        op=mybir.AluOpType.add)
                rz = sm.tile([128, GSZ], F32, tag="rz")
                nc.vector.reciprocal(out=rz[:], in_=Z[:])
                r = sm.tile([128, GSZ, 4], F32, tag="r")
                nc.vector.tensor_tensor(
                    out=r[:], in0=ex[:],
                    in1=rz[:].unsqueeze(2).to_broadcast([128, GSZ, 4]),
                    op=mybir.AluOpType.mult)
                P = sm.tile([128, GSZ, 4, 4], F32, tag="P")
                nc.vector.tensor_tensor(
                    out=P[:],
                    in0=r[:].unsqueeze(3).to_broadcast([128, GSZ, 4, 4]),
                    in1=r[:].unsqueeze(2).to_broadcast([128, GSZ, 4, 4]),
                    op=mybir.AluOpType.mult)
                E = sm.tile([128, GSZ, 4, 4], F32, tag="E")
                nc.scalar.activation(out=E[:], in_=P[:],
                                     func=mybir.ActivationFunctionType.Exp)
                D = sm.tile([128, GSZ, 4], F32, tag="D")
                nc.vector.tensor_reduce(out=D[:], in_=E[:],
                                        axis=mybir.AxisListType.X,
                                        op=mybir.AluOpType.add)
                EN = sm.tile([128, GSZ, 4, 4], F32, tag="EN")
                nc.vector.tensor_tensor(
                    out=EN[:], in0=E[:],
                    in1=r[:].unsqueeze(2).to_broadcast([128, GSZ, 4, 4]),
                    op=mybir.AluOpType.mult)
                Nn = sm.tile([128, GSZ, 4], F32, tag="Nn")
                nc.vector.tensor_reduce(out=Nn[:], in_=EN[:],
                                        axis=mybir.AxisListType.X,
                                        op=mybir.AluOpType.add)
                rD = sm.tile([128, GSZ, 4], F32, tag="rD")
                nc.vector.reciprocal(out=rD[:], in_=D[:])
                nc.vector.tensor_tensor(out=c4[:, g0:g0 + GSZ, :], in0=Nn[:],
                                        in1=rD[:], op=mybir.AluOpType.mult)

                # C build (c4 already slot-ordered -> one TT)
                nc.vector.tensor_tensor(
                    out=C[:, g0:g0 + GSZ, :, :],
                    in0=c4[:, g0:g0 + GSZ, :].to_broadcast([128, GSZ, 4, 64]),
                    in1=m2_sb[:].unsqueeze(1).unsqueeze(1).to_broadcast(
                        [128, GSZ, 4, 64]),
                    op=mybir.AluOpType.mult)

            def emit_builds(g):
                g0 = g * GSZ
                gm = gmid_ps.tile([128, GSZ, 64], F32, tag="gm")
                # m2 / m4 over slots 0-7 then 8-11 (bank split)
                mmat(gm[:, 0:8, :], 1, C[:, g0:g0 + 8, 1, :], True, False)
                mmat(gm[:, 8:12, :], 1, C[:, g0 + 8:g0 + 12, 1, :], True, False)
                mmat(gm[:, 0:8, :], 2, C[:, g0:g0 + 8, 3, :], False, False)
                mmat(gm[:, 8:12, :], 2, C[:, g0 + 8:g0 + 12, 3, :], False, False)
                for c in range(3):
                    phi = phi_of_taul(CLASS_TAUL[c][0])
                    mmat(gm[:, 4 * c:4 * c + 4, :], 3 + phi,
                         C[:, g0 + 4 * c:g0 + 4 * c + 4, 2, :], False, True)
                gl = glow_ps.tile([128, GSZ, 32], F32, tag="gl")
                dn_plan = [(0, 1, 3, 4), (1, 4, 4, 8), (2, 8, 4, 0)]
                for i, (c, o0, on, s0) in enumerate(dn_plan):
                    phi = phi_of_taul(CLASS_TAUL[c][0])
                    mmat(gl[:, o0:o0 + on, :], 9 + phi,
                         C[:, g0 + s0:g0 + s0 + on, 2, 32:64], i == 0, True)
                if g % 2 == 1:
                    mmat(gl[:, 0:1, :], 9 + phi_of_taul(0),
                         C[:, g0 - 12 + 7:g0 - 12 + 8, 2, 32:64], False, True)
                gh = ghigh_ps.tile([128, GSZ, 32], F32, tag="gh")
                up_plan = [(0, 0, 4, 8), (1, 4, 3, 1), (2, 8, 4, 4)]
                for i, (c, o0, on, s0) in enumerate(up_plan):
                    phi = phi_of_taul(CLASS_TAUL[c][0])
                    mmat(gh[:, o0:o0 + on, :], 6 + phi,
                         C[:, g0 + s0:g0 + s0 + on, 2, 0:32], i == 0, True)
                if g % 2 == 0:
                    mmat(gh[:, 7:8, :], 6 + phi_of_taul(11),
                         C[:, g0 + 12:g0 + 13, 2, 0:32], False, True)
                    nc.vector.memset(gl[:, 0, :], 0.0)
                else:
                    nc.vector.memset(gh[:, 7, :], 0.0)
                # assemble Gsb
                nc.vector.scalar_tensor_tensor(
                    out=Gsb[:, g0:g0 + GSZ, 32:96],
                    in0=C[:, g0:g0 + GSZ, 0, :], scalar=0.5, in1=gm[:],
                    op0=mybir.AluOpType.mult, op1=mybir.AluOpType.add)
                nc.scalar.copy(Gsb[:, g0:g0 + GSZ, 0:32], gl[:])
                nc.scalar.copy(Gsb[:, g0:g0 + GSZ, 96:128], gh[:])

            def gsb_idx(row, tt):
                g = 2 * row + tt // GSZ
                return g * GSZ + SLOT_OF[tt % GSZ]

            def emit_big(row, ot_list):
                for ot in ot_list:
                    out2 = out2_ps.tile([128, EMBED], F32, tag="out2")
                    tt_e = 2 * ot
                    if tt_e < 24:
                        nc.tensor.matmul(
                            out=out2[:, :],
                            lhsT=Gsb[:, gsb_idx(row, tt_e), :].bitcast(F32),
                            rhs=X[:, 24 * row + tt_e, 0:EMBED].bitcast(F32),
                            start=True, stop=False, skip_group_check=True)
                    if tt_e - 1 >= 0:
                        nc.tensor.matmul(
                            out=out2[0:64, :],
                            lhsT=Gsb[:, gsb_idx(row, tt_e - 1), 64:128].bitcast(F32),
                            rhs=X[:, 24 * row + tt_e - 1, 0:EMBED].bitcast(F32),
                            start=(tt_e >= 24), stop=True,
                            skip_group_check=True)
                    if tt_e + 1 < 24:
                        # fp32r matmul needs out base_partition 0; this one
                        # targets partitions 64:128 -> plain fp32 (4 cyc/row)
                        nc.tensor.matmul(
                            out=out2[64:128, :],
                            lhsT=Gsb[:, gsb_idx(row, tt_e + 1), 0:64].bitcast(F32),
                            rhs=X[:, 24 * row + tt_e + 1, 0:EMBED].bitcast(F32),
                            start=False, stop=True, skip_group_check=True)
                    osb = outsb_pool.tile([128, EMBED], F32, tag="osb")
                    p0, p1 = (32, 128) if ot == 0 else (0, 32) if ot == 12 \
                        else (0, 128)
                    # copies start at partition 0 (engine partition-base rule);
                    # ot==0 copies garbage rows 0:32 too, DMA skips them.
                    c0, c1 = (0, 32) if ot == 12 else (0, 128)
                    if ot % 2 == 0:
                        nc.vector.tensor_copy(out=osb[c0:c1, :],
                                              in_=out2[c0:c1, :])
                    else:
                        nc.scalar.copy(osb[c0:c1, :], out2[c0:c1, :])
                    base = row * (SEQ // 2) + 128 * ot - 32
                    nc.sync.dma_start(out_d.ap()[base + p0:base + p1, :],
                                      osb[p0:p1, :])

            # ---- staged pipeline ----
            for _rep in range(nrep):
                emit_gather(0)
                emit_gather(1)
                emit_scores(0)
                emit_gather(2)
                emit_scores(1)
                emit_builds(0)
                emit_big(0, list(range(0, 6)))
                emit_gather(3)
                emit_scores(2)
                emit_builds(1)
                emit_big(0, list(range(6, 13)))
                emit_scores(3)
                emit_builds(2)
                emit_big(1, list(range(0, 6)))
                emit_builds(3)
                emit_big(1, list(range(6, 13)))

    return nc


_CACHE = {}


def _get_nc(nrep=1):
    key = f"nc{nrep}"
    if key not in _CACHE:
        nc = bacc.Bacc("TRN2", target_bir_lowering=False, debug=False)
        emit_program(nc, nrep=nrep)
        nc.compile()
        _CACHE[key] = nc
    return _CACHE[key]


def prepare_in_maps(input_ids, embed_table, w_score, b_score=None):
    # b_score only shifts all 4 scores equally -> softmax-invariant; unused.
    t4pad = build_t4pad(embed_table, w_score)
    smats = build_smats()
    m2 = build_m2mask()
    idx_streams = build_idx_streams(input_ids)
    return [{"t4pad": t4pad, "idxs": idx_streams[core],
             "smats": smats, "m2m": m2} for core in range(NCORES)]


def assemble_out(results):
    outs = [results[c]["out"].reshape(BLOC, SEQ // 2, EMBED)
            for c in range(NCORES)]
    return np.concatenate(outs, axis=0)


def kernel(input_ids, embed_table, w_score, b_score):
    in_maps = prepare_in_maps(input_ids, embed_table, w_score, b_score)
    res = run_bass_kernel_spmd(_get_nc(), in_maps,
                               core_ids=list(range(NCORES)))
    return assemble_out(res.results)



# revision 16
# speedup vs baseline: 1.4531x; 1.4531x over previous
"""GBST Trainium2 kernel (nn_GBST_42434276884940).

Self-contained: takes FULL inputs, shards batch over 8 NeuronCores
(2 rows/core), runs a Bass/Tile kernel per core, gathers full output.

Device algorithm per core (6144 positions = 48 chunks of 128):
- Embedding gather as one-hot matmuls on the PE (the indirect-DMA
  gather is serialized on the single gpsimd software DGE queue and
  dominates runtime): per chunk, DVE builds a one-hot [128v, 2, 128pos]
  bf16 mask by comparing a partition-replicated index row against a
  per-partition iota, then 2 accumulating bf16 matmuls against the
  resident augmented table [v, 258] = [emb (256), emb@w (1), 0] produce
  X_psum [128pos, 258] fp32. The Pool engine adds the periodic
  positional encoding (phase = p%4 is constant per partition) during the
  PSUM->SBUF cast to bf16.
- Scores: pooled per-position scores for block sizes 1..4 via small PE
  matmuls with constant block-pooling matrices, batched over group
  PAIRS (24 slots per matmul, 2-level APs); softmax + the tiny
  self-attention calibration as elementwise DVE/ACT ops on [128,24,4]
  tiles -> combine weights c4 (bf16).
- The pool->repeat->weighted-combine->downsample(2) stack is one banded
  linear operator G. Its 128x128 chunk tiles (lhsT "Gsb", bf16) are
  built on the PE from C[l, j, m] = c4 * m2-mask products, then applied:
  out2[t, h] = sum_k Gsb[k, t] * X[k, h], accumulated in PSUM fp32 and
  DMA'd to DRAM directly from PSUM.
All matmuls bf16 (1 cyc/col vs fp32's 4); fp32 accumulation in PSUM.
"""

import sys

import numpy as np

if "/opt/trn_rl_repo" not in sys.path:
    sys.path.insert(0, "/opt/trn_rl_repo")

import concourse.bass as bass
import concourse.tile as tile
from concourse import bacc, mybir
from concourse.bass_utils import run_bass_kernel_spmd

F32 = mybir.dt.float32
BF16 = mybir.dt.bfloat16
NPBF16 = np.dtype(mybir.dt.np(mybir.dt.bfloat16))

MAX_BLOCK = 4
EMBED = 256
VOCAB = 256
BATCH = 16
SEQ = 3072
NCORES = 8
BLOC = BATCH // NCORES           # 2
NPOS = BLOC * SEQ                # 6144
NCHUNK = NPOS // 128             # 48
NGROUP = 4
GSZ = NCHUNK // NGROUP           # 12
ELEM = 258                       # 256 emb + 1 score + 1 pad (even)

SLOTS = [0, 3, 6, 9, 2, 5, 8, 11, 1, 4, 7, 10]   # slot s -> tau_l
SLOT_OF = {t: s for s, t in enumerate(SLOTS)}
CLASS_TAUL = [[0, 3, 6, 9], [2, 5, 8, 11], [1, 4, 7, 10]]


# ---------------------------------------------------------------- host consts

def _sinusoidal_pe(max_len, d):
    pos = np.arange(max_len, dtype=np.float32)[:, None]
    div = np.exp(np.arange(0, d, 2, dtype=np.float32) * (-np.log(10000.0) / d))
    pe = np.zeros((max_len, d), dtype=np.float32)
    pe[:, 0::2] = np.sin(pos * div)
    pe[:, 1::2] = np.cos(pos * div)
    return pe


def build_taug(embed_table, w_score):
    """bf16 [128, 2, ELEM]: row v of slice s = table[128s+v] ++ table@w."""
    table = np.asarray(embed_table, dtype=np.float32)
    w = np.asarray(w_score, dtype=np.float32).reshape(EMBED)
    t = np.zeros((VOCAB, ELEM), dtype=np.float32)
    t[:, :EMBED] = table
    t[:, EMBED] = table @ w
    return np.ascontiguousarray(
        t.reshape(2, 128, ELEM).transpose(1, 0, 2)).astype(NPBF16)


def build_pe258(w_score):
    """fp32 [128, ELEM]: row p = pe[p%4] ++ pe[p%4]@w (phase is per-partition
    because 128 % 4 == 0)."""
    w = np.asarray(w_score, dtype=np.float32).reshape(EMBED)
    pe = _sinusoidal_pe(MAX_BLOCK, EMBED)
    out = np.zeros((128, ELEM), dtype=np.float32)
    for p in range(128):
        out[p, :EMBED] = pe[p % 4]
        out[p, EMBED] = pe[p % 4] @ w
    return out


def build_iota2():
    """bf16 [128, 2, 128]: value p + 128*s, materialized along pos axis."""
    p = np.arange(128, dtype=np.float32)
    v = p[:, None] + 128.0 * np.arange(2, dtype=np.float32)[None, :]
    return np.repeat(v[:, :, None], 128, axis=2).astype(NPBF16)


def build_oh4():
    """bf16 [4, 128]: onehot of pos%4 (lhsT of the pe-add matmul)."""
    return (np.arange(128)[None, :] % 4 ==
            np.arange(4)[:, None]).astype(NPBF16)


def build_peaug(w_score):
    """bf16 [4, ELEM]: pe row phi ++ pe[phi]@w (rhs of the pe-add matmul)."""
    w = np.asarray(w_score, dtype=np.float32).reshape(EMBED)
    pe = _sinusoidal_pe(MAX_BLOCK, EMBED)
    out = np.zeros((4, ELEM), dtype=np.float32)
    out[:, :EMBED] = pe
    out[:, EMBED] = pe @ w
    return out.astype(NPBF16)


def phi_of_taul(tau_l):
    return (2 * tau_l) % 3


def build_smats():
    k = np.arange(128)
    mats = np.zeros((12, 128, 128), dtype=np.float32)
    mats[0] = 0.5 * np.eye(128, dtype=np.float32)
    mats[1] = 0.25 * (k[:, None] // 2 == k[None, :] // 2)
    mats[2] = 0.125 * (k[:, None] // 4 == k[None, :] // 4)
    for phi in range(3):
        mats[3 + phi] = (1 / 6) * ((k[:, None] + phi) // 3 == (k[None, :] + phi) // 3)
        mats[6 + phi] = (1 / 6) * ((128 + k[:, None] + phi) // 3 == (k[None, :] + phi) // 3)
        mats[9 + phi] = (1 / 6) * ((k[:, None] - 128 + phi) // 3 == (k[None, :] + phi) // 3)
    return mats.astype(NPBF16)


def build_m2b():
    """bf16 [128, 64, 4]: m2 mask replicated over the trailing m axis."""
    p = np.arange(128)
    j = np.arange(64)
    m2 = (j[None, :] == p[:, None] // 2).astype(np.float32)
    return np.repeat(m2[:, :, None], 4, axis=2).astype(NPBF16)


def build_idx_streams(input_ids):
    """Per-core bf16 [NCHUNK, 128]: vocab id of position g = 128*c + p."""
    ids = np.asarray(input_ids).astype(np.int64)
    out = []
    g = np.arange(NPOS)
    row, l = g // SEQ, g % SEQ
    for core in range(NCORES):
        core_ids = ids[core * BLOC:(core + 1) * BLOC]
        vid = core_ids[row, l].astype(np.float32)
        out.append(vid.reshape(NCHUNK, 128).astype(NPBF16))
    return out


# ---------------------------------------------------------------- device prog

def _ap2(base_ap, extra_off, dims):
    """Raw AP: base partition dim + given free [stride, size] dims."""
    return bass.AP(tensor=base_ap.tensor, offset=base_ap.offset + extra_off,
                   ap=[list(base_ap.ap[0])] + [list(d) for d in dims])


def emit_program(nc, nrep=1):
    idxs_d = nc.dram_tensor("idxs", [NCHUNK, 128], BF16, kind="ExternalInput")
    taug_d = nc.dram_tensor("taug", [128, 2, ELEM], BF16, kind="ExternalInput")
    oh4_d = nc.dram_tensor("oh4", [4, 128], BF16, kind="ExternalInput")
    peaug_d = nc.dram_tensor("peaug", [4, ELEM], BF16, kind="ExternalInput")
    iota2_d = nc.dram_tensor("iota2", [128, 2, 128], BF16, kind="ExternalInput")
    smats_d = nc.dram_tensor("smats", [12, 128, 128], BF16, kind="ExternalInput")
    m2b_d = nc.dram_tensor("m2b", [128, 64, 4], BF16, kind="ExternalInput")
    out_d = nc.dram_tensor("out", [BLOC * SEQ // 2, EMBED], F32,
                           kind="ExternalOutput")

    with tile.TileContext(nc) as tc:
        with (
            tc.tile_pool(name="consts", bufs=1) as consts,
            tc.tile_pool(name="big", bufs=1) as big,
            tc.tile_pool(name="oh", bufs=3) as oh_pool,
            tc.tile_pool(name="sm", bufs=2) as sm,
            tc.tile_pool(name="outsb", bufs=4) as outsb_pool,
            tc.tile_pool(name="emb_ps", bufs=2, space="PSUM") as emb_ps,
            tc.tile_pool(name="gmid_ps", bufs=1, space="PSUM") as gmid_ps,
            tc.tile_pool(name="glow_ps", bufs=1, space="PSUM") as glow_ps,
            tc.tile_pool(name="ghigh_ps", bufs=1, space="PSUM") as ghigh_ps,
            tc.tile_pool(name="out2_ps", bufs=2, space="PSUM") as out2_ps,
        ):
            # ---- constants to SBUF ----
            smats_sb = consts.tile([128, 12, 128], BF16, tag="smats")
            nc.sync.dma_start(
                smats_sb[:],
                bass.AP(tensor=smats_d, offset=0,
                        ap=[[128, 128], [128 * 128, 12], [1, 128]]))
            taug_sb = consts.tile([128, 2, ELEM], BF16, tag="taug")
            nc.sync.dma_start(taug_sb[:], taug_d.ap()[:, :, :])
            oh4_sb = consts.tile([4, 128], BF16, tag="oh4")
            nc.sync.dma_start(oh4_sb[:], oh4_d.ap()[:, :])
            peaug_sb = consts.tile([4, ELEM], BF16, tag="peaug")
            nc.sync.dma_start(peaug_sb[:], peaug_d.ap()[:, :])
            iota_sb = consts.tile([128, 2, 128], BF16, tag="iota")
            nc.sync.dma_start(iota_sb[:], iota2_d.ap()[:, :, :])
            m2b_sb = consts.tile([128, 64, 4], BF16, tag="m2b")
            nc.sync.dma_start(m2b_sb[:], m2b_d.ap()[:, :, :])
            # idx rows replicated to all partitions via 0-stride source AP,
            # one DMA per group so group 0 compute starts early
            idxr_sb = consts.tile([128, NCHUNK, 128], BF16, tag="idxr")
            for g in range(NGROUP):
                nc.sync.dma_start(
                    idxr_sb[:, g * GSZ:(g + 1) * GSZ, :],
                    bass.AP(tensor=idxs_d, offset=g * GSZ * 128,
                            ap=[[0, 128], [128, GSZ], [1, 128]]))

            # ---- persistent big tensors ----
            X = big.tile([128, NCHUNK, ELEM], BF16, tag="X")
            C = big.tile([128, NCHUNK, 64, 4], BF16, tag="C")
            Gsb = big.tile([128, NCHUNK, 128], BF16, tag="Gsb")
            d2 = big.tile([128, NCHUNK], BF16, tag="d2")      # slot-ordered
            c4 = big.tile([128, NCHUNK, 4], BF16, tag="c4")   # slot-ordered

            def mmat(out_ap, mi, rhs_ap, start, stop):
                nc.tensor.matmul(out=out_ap, lhsT=smats_sb[:, mi, :],
                                 rhs=rhs_ap, start=start, stop=stop,
                                 skip_group_check=True)

            def emit_embed(g):
                g0 = g * GSZ
                for c in range(GSZ):
                    ch = g0 + c
                    oht = oh_pool.tile([128, 2, 128], BF16, tag="oh")
                    nc.vector.tensor_tensor(
                        out=oht[:],
                        in0=idxr_sb[:, ch, :].unsqueeze(1).to_broadcast(
                            [128, 2, 128]),
                        in1=iota_sb[:],
                        op=mybir.AluOpType.is_equal)
                    ps = emb_ps.tile([128, ELEM], F32, tag="emb",
                                     padded_shape=[128, 512])
                    nc.tensor.matmul(out=ps[:], lhsT=oh4_sb[:],
                                     rhs=peaug_sb[:],
                                     start=True, stop=False,
                                     skip_group_check=True)
                    nc.tensor.matmul(out=ps[:], lhsT=oht[:, 0, :],
                                     rhs=taug_sb[:, 0, :],
                                     start=False, stop=False,
                                     skip_group_check=True)
                    nc.tensor.matmul(out=ps[:], lhsT=oht[:, 1, :],
                                     rhs=taug_sb[:, 1, :],
                                     start=False, stop=True,
                                     skip_group_check=True)
                    # ACT: X = psum (cast to bf16); pe already accumulated
                    nc.scalar.copy(X[:, ch, :], ps[:])
                # d2 = 2*z, written in slot order: class c block <- tau_l
                # stride-3 run
                for c in range(3):
                    t0 = CLASS_TAUL[c][0]
                    nc.scalar.mul(
                        d2[:, g0 + 4 * c:g0 + 4 * c + 4].unsqueeze(2),
                        X[:, g0 + t0:g0 + t0 + 10:3, EMBED:EMBED + 1], 2.0)

            def emit_scores(ga):
                """Scores for the self-contained group pair (ga, ga+1):
                 24 slots per matmul via [group(12), run] 2-level APs."""
                a0 = ga * GSZ
                W = 2 * GSZ  # 24
                # scT [128, 4, W] carved out of the gm bank (disjoint
                # lifetimes; Tile serializes the aliased uses)
                scT_full = gmid_ps.tile([128, GSZ, 64], F32, tag="gm")
                sc_b = scT_full[:]
                d2_b = d2[:]

                def sc_row(m, o0, on):
                    return _ap2(sc_b, m * W + o0, [[1, on]])

                def sc_ap(m, o0, on):
                    return _ap2(sc_b, m * W + o0, [[GSZ, 2], [1, on]])

                def d2_ap(s0, on):
                    return _ap2(d2_b, a0 + s0, [[GSZ, 2], [1, on]])

                mmat(sc_row(0, 0, W), 0, d2[:, a0:a0 + W], True, False)
                mmat(sc_row(1, 0, W), 1, d2[:, a0:a0 + W], False, False)
                mmat(sc_row(3, 0, W), 2, d2[:, a0:a0 + W], False, False)
                for c in range(3):
                    phi = phi_of_taul(CLASS_TAUL[c][0])
                    mmat(sc_ap(2, 4 * c, 4), 3 + phi, d2_ap(4 * c, 4),
                         False, False)
                up_sc = [(0, 0, 4, 8), (1, 4, 3, 1), (2, 8, 4, 4)]
                dn_sc = [(0, 1, 3, 4), (1, 4, 4, 8), (2, 8, 4, 0)]
                for plan, base in ((up_sc, 6), (dn_sc, 9)):
                    for c, o0, on, s0 in plan:
                        phi = phi_of_taul(CLASS_TAUL[c][0])
                        mmat(sc_ap(2, o0, on), base + phi, d2_ap(s0, on),
                             False, False)
                # pair-internal boundary fixes: ga is even (up-fix slot 7 <-
                # gb slot 0), gb odd (dn-fix slot 0 <- ga slot 7)
                mmat(sc_row(2, 7, 1), 6 + phi_of_taul(11),
                     d2[:, a0 + GSZ:a0 + GSZ + 1], False, False)
                mmat(sc_row(2, GSZ, 1), 9 + phi_of_taul(0),
                     d2[:, a0 + 7:a0 + 8], False, True)

                # softmax + calibration (scS transposed to [128, slot, m])
                scS = sm.tile([128, W, 4], F32, tag="scS")
                scT_t = _ap2(sc_b, 0, [[1, W], [W, 4]])
                nc.vector.tensor_copy(out=scS[:], in_=scT_t)
                ex = sm.tile([128, W, 4], F32, tag="ex")
                nc.scalar.activation(out=ex[:], in_=scS[:],
                                     func=mybir.ActivationFunctionType.Exp)
                Z = sm.tile([128, W], F32, tag="Z")
                nc.vector.tensor_reduce(out=Z[:], in_=ex[:],
                                        axis=mybir.AxisListType.X,
                                        op=mybir.AluOpType.add)
                rz = sm.tile([128, W], F32, tag="rz")
                nc.vector.reciprocal(out=rz[:], in_=Z[:])
                r = sm.tile([128, W, 4], F32, tag="r")
                nc.vector.tensor_tensor(
                    out=r[:], in0=ex[:],
                    in1=rz[:].unsqueeze(2).to_broadcast([128, W, 4]),
                    op=mybir.AluOpType.mult)
                P = sm.tile([128, W, 4, 4], F32, tag="P")
                nc.vector.tensor_tensor(
                    out=P[:],
                    in0=r[:].unsqueeze(3).to_broadcast([128, W, 4, 4]),
                    in1=r[:].unsqueeze(2).to_broadcast([128, W, 4, 4]),
                    op=mybir.AluOpType.mult)
                E = sm.tile([128, W, 4, 4], F32, tag="E")
                nc.scalar.activation(out=E[:], in_=P[:],
                                     func=mybir.ActivationFunctionType.Exp)
                D = sm.tile([128, W, 4], F32, tag="D")
                nc.vector.tensor_reduce(out=D[:], in_=E[:],
                                        axis=mybir.AxisListType.X,
                                        op=mybir.AluOpType.add)
                EN = sm.tile([128, W, 4, 4], F32, tag="EN")
                nc.vector.tensor_tensor(
                    out=EN[:], in0=E[:],
                    in1=r[:].unsqueeze(2).to_broadcast([128, W, 4, 4]),
                    op=mybir.AluOpType.mult)
                Nn = sm.tile([128, W, 4], F32, tag="Nn")
                nc.vector.tensor_reduce(out=Nn[:], in_=EN[:],
                                        axis=mybir.AxisListType.X,
                                        op=mybir.AluOpType.add)
                rD = sm.tile([128, W, 4], F32, tag="rD")
                nc.vector.reciprocal(out=rD[:], in_=D[:])
                nc.vector.tensor_tensor(out=c4[:, a0:a0 + W, :], in0=Nn[:],
                                        in1=rD[:], op=mybir.AluOpType.mult)

                # C[l, j, m] = c4[l, m] * m2[l, j] (both bf16, packed last)
                nc.vector.tensor_tensor(
                    out=C[:, a0:a0 + W, :, :],
                    in0=c4[:, a0:a0 + W, :].unsqueeze(2).to_broadcast(
                        [128, W, 64, 4]),
                    in1=m2b_sb[:].unsqueeze(1).to_broadcast([128, W, 64, 4]),
                    op=mybir.AluOpType.mult)

            def emit_builds(g):
                g0 = g * GSZ
                gm = gmid_ps.tile([128, GSZ, 64], F32, tag="gm")
                # m2 / m4 over slots 0-7 then 8-11 (bank split)
                mmat(gm[:, 0:8, :], 1, C[:, g0:g0 + 8, :, 1], True, False)
                mmat(gm[:, 8:12, :], 1, C[:, g0 + 8:g0 + 12, :, 1], True, False)
                mmat(gm[:, 0:8, :], 2, C[:, g0:g0 + 8, :, 3], False, False)
                mmat(gm[:, 8:12, :], 2, C[:, g0 + 8:g0 + 12, :, 3], False, False)
                for c in range(3):
                    phi = phi_of_taul(CLASS_TAUL[c][0])
                    mmat(gm[:, 4 * c:4 * c + 4, :], 3 + phi,
                         C[:, g0 + 4 * c:g0 + 4 * c + 4, :, 2], False, True)
                gl = glow_ps.tile([128, GSZ, 32], F32, tag="gl")
                dn_plan = [(0, 1, 3, 4), (1, 4, 4, 8), (2, 8, 4, 0)]
                for i, (c, o0, on, s0) in enumerate(dn_plan):
                    phi = phi_of_taul(CLASS_TAUL[c][0])
                    mmat(gl[:, o0:o0 + on, :], 9 + phi,
                         C[:, g0 + s0:g0 + s0 + on, 32:64, 2], i == 0, True)
                if g % 2 == 1:
                    mmat(gl[:, 0:1, :], 9 + phi_of_taul(0),
                         C[:, g0 - 12 + 7:g0 - 12 + 8, 32:64, 2], False, True)
                gh = ghigh_ps.tile([128, GSZ, 32], F32, tag="gh")
                up_plan = [(0, 0, 4, 8), (1, 4, 3, 1), (2, 8, 4, 4)]
                for i, (c, o0, on, s0) in enumerate(up_plan):
                    phi = phi_of_taul(CLASS_TAUL[c][0])
                    mmat(gh[:, o0:o0 + on, :], 6 + phi,
                         C[:, g0 + s0:g0 + s0 + on, 0:32, 2], i == 0, True)
                if g % 2 == 0:
                    mmat(gh[:, 7:8, :], 6 + phi_of_taul(11),
                         C[:, g0 + 12:g0 + 13, 0:32, 2], False, True)
                    nc.vector.memset(gl[:, 0, :], 0.0)
                else:
                    nc.vector.memset(gh[:, 7, :], 0.0)
                # assemble Gsb (bf16)
                nc.vector.scalar_tensor_tensor(
                    out=Gsb[:, g0:g0 + GSZ, 32:96],
                    in0=C[:, g0:g0 + GSZ, :, 0], scalar=0.5, in1=gm[:],
                    op0=mybir.AluOpType.mult, op1=mybir.AluOpType.add)
                nc.scalar.copy(Gsb[:, g0:g0 + GSZ, 0:32], gl[:])
                nc.scalar.copy(Gsb[:, g0:g0 + GSZ, 96:128], gh[:])

            def gsb_idx(row, tt):
                g = 2 * row + tt // GSZ
                return g * GSZ + SLOT_OF[tt % GSZ]

            def emit_big(row, ot_list):
                for ot in ot_list:
                    out2 = out2_ps.tile([128, EMBED], F32, tag="out2")
                    tt_e = 2 * ot
                    if tt_e < 24:
                        nc.tensor.matmul(
                            out=out2[:, :],
                            lhsT=Gsb[:, gsb_idx(row, tt_e), :],
                            rhs=X[:, 24 * row + tt_e, 0:EMBED],
                            start=True, stop=False, skip_group_check=True)
                    if tt_e - 1 >= 0:
                        nc.tensor.matmul(
                            out=out2[0:64, :],
                            lhsT=Gsb[:, gsb_idx(row, tt_e - 1), 64:128],
                            rhs=X[:, 24 * row + tt_e - 1, 0:EMBED],
                            start=(tt_e >= 24), stop=True,
                            skip_group_check=True)
                    if tt_e + 1 < 24:
                        nc.tensor.matmul(
                            out=out2[64:128, :],
                            lhsT=Gsb[:, gsb_idx(row, tt_e + 1), 0:64],
                            rhs=X[:, 24 * row + tt_e + 1, 0:EMBED],
                            start=False, stop=True, skip_group_check=True)
                    osb = outsb_pool.tile([128, EMBED], F32, tag="osb")
                    base = row * (SEQ // 2) + 128 * ot - 32
                    p0, p1 = (32, 128) if ot == 0 else (0, 32) if ot == 12 \
                        else (0, 128)
                    # copies start at partition 0 (engine partition-base
                    # rule); ot==0 copies garbage rows 0:32, DMA skips them
                    c0, c1 = (0, 32) if ot == 12 else (0, 128)
                    if ot % 2 == 0:
                        nc.vector.tensor_copy(out=osb[c0:c1, :],
                                              in_=out2[c0:c1, :])
                    else:
                        nc.scalar.copy(osb[c0:c1, :], out2[c0:c1, :])
                    nc.sync.dma_start(out_d.ap()[base + p0:base + p1, :],
                                      osb[p0:p1, :])

            # ---- staged pipeline ----
            for _rep in range(nrep):
                emit_embed(0)
                emit_embed(1)
                emit_embed(2)
                emit_scores(0)
                emit_builds(0)
                emit_big(0, list(range(0, 6)))
                emit_embed(3)
                emit_builds(1)
                emit_big(0, list(range(6, 13)))
                emit_scores(2)
                emit_builds(2)
                emit_big(1, list(range(0, 6)))
                emit_builds(3)
                emit_big(1, list(range(6, 13)))

    return nc


_CACHE = {}


def _get_nc(nrep=1):
    key = f"nc{nrep}"
    if key not in _CACHE:
        nc = bacc.Bacc("TRN2", target_bir_lowering=False, debug=False)
        emit_program(nc, nrep=nrep)
        nc.compile()
        _CACHE[key] = nc
    return _CACHE[key]


def prepare_in_maps(input_ids, embed_table, w_score, b_score=None):
    # b_score only shifts all 4 scores equally -> softmax-invariant; unused.
    taug = build_taug(embed_table, w_score)
    oh4 = build_oh4()
    peaug = build_peaug(w_score)
    iota2 = build_iota2()
    smats = build_smats()
    m2b = build_m2b()
    idx_streams = build_idx_streams(input_ids)
    return [{"idxs": idx_streams[core], "taug": taug, "oh4": oh4,
             "peaug": peaug, "iota2": iota2, "smats": smats, "m2b": m2b}
            for core in range(NCORES)]


def assemble_out(results):
    outs = [results[c]["out"].reshape(BLOC, SEQ // 2, EMBED)
            for c in range(NCORES)]
    return np.concatenate(outs, axis=0)


def kernel(input_ids, embed_table, w_score, b_score):
    in_maps = prepare_in_maps(input_ids, embed_table, w_score, b_score)
    res = run_bass_kernel_spmd(_get_nc(), in_maps,
                               core_ids=list(range(NCORES)))
    return assemble_out(res.results)


# revision 29
# speedup vs baseline: 2.5408x; 1.7486x over previous
"""GBST Trainium2 kernel (nn_GBST_42434276884940).

Self-contained: takes FULL inputs, shards batch over 8 NeuronCores
(2 rows/core), runs a Bass/Tile kernel per core, gathers full output.

Device algorithm per core (6144 positions = 48 chunks of 128):
- Embedding gather as one-hot matmuls on the PE (an indirect-DMA gather
  serializes on the single gpsimd software DGE queue and dominates
  runtime): per chunk, DVE builds a one-hot [128v, 2, 128pos] bf16 mask
  by comparing a partition-replicated index row against a per-partition
  iota, then 2 accumulating bf16 matmuls against the resident augmented
  table [v, 258] = [emb (256), 2*emb@w (1), 0] produce X_psum
  [128pos, 258] fp32. A third K=4 matmul against constant [pos%4 one-hot,
  pe rows] adds the periodic positional encoding in PSUM. ACT casts
  PSUM->SBUF bf16 (a few chunks go via DVE with the pe-add fused there
  instead, for engine balance).
- Scores: pooled per-position scores for block sizes 1..4 via small PE
  matmuls with constant block-pooling matrices, batched over group
  PAIRS; the rhs reads X's score column directly with strided APs (the
  x2 score scale is baked into the table). Softmax + the tiny
  self-attention calibration run as elementwise DVE/ACT ops on
  [128,24,4] tiles -> combine weights c4 (bf16). The Pool engine
  expands c4 into the C mask products.
- The pool->repeat->weighted-combine->downsample(2) stack is one banded
  linear operator G. Its 128x128 chunk tiles (lhsT "Gsb", bf16) are
  built on the PE from C, then applied: out2[t, h] = sum_k Gsb[k, t] *
  X[k, h], accumulated in PSUM fp32, staged to SBUF (DVE/ACT
  alternating), and DMA'd out in 256-row pairs.
All matmuls bf16 (1 cyc/col vs fp32's 4); fp32 accumulation in PSUM.
Constants arrive as one packed bf16 DMA + smats + replicated idx rows
(0-stride source AP), split across the SP and ACT HWDGE queues.
"""

import sys

import numpy as np

if "/opt/trn_rl_repo" not in sys.path:
    sys.path.insert(0, "/opt/trn_rl_repo")

import concourse.bass as bass
import concourse.tile as tile
from concourse import bacc, mybir
from concourse.bass_utils import run_bass_kernel_spmd

F32 = mybir.dt.float32
BF16 = mybir.dt.bfloat16
NPBF16 = np.dtype(mybir.dt.np(mybir.dt.bfloat16))

MAX_BLOCK = 4
EMBED = 256
VOCAB = 256
BATCH = 16
SEQ = 3072
NCORES = 8
BLOC = BATCH // NCORES           # 2
NPOS = BLOC * SEQ                # 6144
NCHUNK = NPOS // 128             # 48
NGROUP = 4
GSZ = NCHUNK // NGROUP           # 12
ELEM = 258                       # 256 emb + 1 score + 1 pad (even)

SLOTS = [0, 3, 6, 9, 2, 5, 8, 11, 1, 4, 7, 10]   # slot s -> tau_l
SLOT_OF = {t: s for s, t in enumerate(SLOTS)}
CLASS_TAUL = [[0, 3, 6, 9], [2, 5, 8, 11], [1, 4, 7, 10]]

# packed bf16 const columns
O_TAUG = 0            # [128, 2, 258]
O_IOTA = 516          # [128, 2, 128]
O_M2B = 772           # [128, 64, 4]
O_OH4 = 1028          # [4, 128]
O_PEAUG = 1156        # [4, 258]
NCONST = 1414

DVE_XCOPY = 6         # chunks ch % DVE_XCOPY == DVE_XCOPY-1 copy via DVE


# ---------------------------------------------------------------- host consts

def _sinusoidal_pe(max_len, d):
    pos = np.arange(max_len, dtype=np.float32)[:, None]
    div = np.exp(np.arange(0, d, 2, dtype=np.float32) * (-np.log(10000.0) / d))
    pe = np.zeros((max_len, d), dtype=np.float32)
    pe[:, 0::2] = np.sin(pos * div)
    pe[:, 1::2] = np.cos(pos * div)
    return pe


def build_consts(embed_table, w_score):
    """Packed bf16 [128, NCONST]: taug, iota2, m2b, oh4, peaug.
    Score columns carry 2*(row@w) so X[:,:,256] is the d2 the score
    matmuls consume directly."""
    table = np.asarray(embed_table, dtype=np.float32)
    w = np.asarray(w_score, dtype=np.float32).reshape(EMBED)
    pe = _sinusoidal_pe(MAX_BLOCK, EMBED)
    out = np.zeros((128, NCONST), dtype=np.float32)
    # taug [128, 2, 258]
    t = np.zeros((VOCAB, ELEM), dtype=np.float32)
    t[:, :EMBED] = table
    t[:, EMBED] = 2.0 * (table @ w)
    out[:, O_TAUG:O_TAUG + 516] = t.reshape(2, 128, ELEM).transpose(
        1, 0, 2).reshape(128, 516)
    # iota2 [128, 2, 128] value p + 128*s, constant along pos
    p = np.arange(128, dtype=np.float32)
    v = p[:, None] + 128.0 * np.arange(2, dtype=np.float32)[None, :]
    out[:, O_IOTA:O_IOTA + 256] = np.repeat(
        v[:, :, None], 128, axis=2).reshape(128, 256)
    # m2b [128, 64, 4]
    j = np.arange(64)
    m2 = (j[None, :] == np.arange(128)[:, None] // 2).astype(np.float32)
    out[:, O_M2B:O_M2B + 256] = np.repeat(
        m2[:, :, None], 4, axis=2).reshape(128, 256)
    # oh4 [4, 128] (rows 0:4)
    out[0:4, O_OH4:O_OH4 + 128] = (
        np.arange(128)[None, :] % 4 == np.arange(4)[:, None])
    # peaug [4, 258] (rows 0:4)
    out[0:4, O_PEAUG:O_PEAUG + EMBED] = pe
    out[0:4, O_PEAUG + EMBED] = 2.0 * (pe @ w)
    return out.astype(NPBF16)


def build_pe258(w_score):
    """fp32 [128, ELEM]: row p = pe[p%4] ++ 2*pe[p%4]@w (phase is
    per-partition because 128 % 4 == 0); used by DVE-fused X copies."""
    w = np.asarray(w_score, dtype=np.float32).reshape(EMBED)
    pe = _sinusoidal_pe(MAX_BLOCK, EMBED)
    out = np.zeros((128, ELEM), dtype=np.float32)
    for p in range(128):
        out[p, :EMBED] = pe[p % 4]
        out[p, EMBED] = 2.0 * (pe[p % 4] @ w)
    return out


def phi_of_taul(tau_l):
    return (2 * tau_l) % 3


def build_smats():
    k = np.arange(128)
    mats = np.zeros((12, 128, 128), dtype=np.float32)
    mats[0] = 0.5 * np.eye(128, dtype=np.float32)
    mats[1] = 0.25 * (k[:, None] // 2 == k[None, :] // 2)
    mats[2] = 0.125 * (k[:, None] // 4 == k[None, :] // 4)
    for phi in range(3):
        mats[3 + phi] = (1 / 6) * ((k[:, None] + phi) // 3 == (k[None, :] + phi) // 3)
        mats[6 + phi] = (1 / 6) * ((128 + k[:, None] + phi) // 3 == (k[None, :] + phi) // 3)
        mats[9 + phi] = (1 / 6) * ((k[:, None] - 128 + phi) // 3 == (k[None, :] + phi) // 3)
    return mats.astype(NPBF16)


def build_idx_streams(input_ids):
    """Per-core bf16 [NCHUNK, 128]: vocab id of position g = 128*c + p."""
    ids = np.asarray(input_ids).astype(np.int64)
    out = []
    g = np.arange(NPOS)
    row, l = g // SEQ, g % SEQ
    for core in range(NCORES):
        core_ids = ids[core * BLOC:(core + 1) * BLOC]
        vid = core_ids[row, l].astype(np.float32)
        out.append(vid.reshape(NCHUNK, 128).astype(NPBF16))
    return out


# ---------------------------------------------------------------- device prog

def _ap2(base_ap, extra_off, dims):
    """Raw AP: base partition dim + given free [stride, size] dims."""
    return bass.AP(tensor=base_ap.tensor, offset=base_ap.offset + extra_off,
                   ap=[list(base_ap.ap[0])] + [list(d) for d in dims])


def emit_program(nc, nrep=1):
    idxs_d = nc.dram_tensor("idxs", [NCHUNK, 128], BF16, kind="ExternalInput")
    consts_d = nc.dram_tensor("kconsts", [128, NCONST], BF16,
                              kind="ExternalInput")
    pe258_d = nc.dram_tensor("pe258", [128, ELEM], F32, kind="ExternalInput")
    smats_d = nc.dram_tensor("smats", [12, 128, 128], BF16,
                             kind="ExternalInput")
    out_d = nc.dram_tensor("out", [BLOC * SEQ // 2, EMBED], F32,
                           kind="ExternalOutput")

    with tile.TileContext(nc) as tc:
        with (
            tc.tile_pool(name="consts", bufs=1) as consts,
            tc.tile_pool(name="big", bufs=1) as big,
            tc.tile_pool(name="oh", bufs=18) as oh_pool,
            tc.tile_pool(name="sm", bufs=2) as sm,
            tc.tile_pool(name="outsb", bufs=4) as outsb_pool,
            tc.tile_pool(name="emb_ps", bufs=2, space="PSUM") as emb_ps,
            tc.tile_pool(name="gmid_ps", bufs=1, space="PSUM") as gmid_ps,
            tc.tile_pool(name="glow_ps", bufs=1, space="PSUM") as glow_ps,
            tc.tile_pool(name="ghigh_ps", bufs=1, space="PSUM") as ghigh_ps,
            tc.tile_pool(name="out2_ps", bufs=2, space="PSUM") as out2_ps,
        ):
            # ---- ACT function-table prewarm (off the critical path) ----
            warm = sm.tile([128, 4], F32, tag="warm")
            nc.vector.memset(warm[:], 0.0)
            nc.scalar.activation(out=warm[:], in_=warm[:],
                                 func=mybir.ActivationFunctionType.Exp)

            # ---- constants to SBUF (SP queue: idx rows + smats; ACT
            # queue: packed consts + pe258) ----
            idxr_sb = consts.tile([128, NCHUNK, 128], BF16, tag="idxr")
            # idx rows replicated to all partitions via 0-stride source AP,
            # one DMA per group so group 0 compute starts early
            for g in range(NGROUP):
                nc.sync.dma_start(
                    idxr_sb[:, g * GSZ:(g + 1) * GSZ, :],
                    bass.AP(tensor=idxs_d, offset=g * GSZ * 128,
                            ap=[[0, 128], [128, GSZ], [1, 128]]))
            kc_sb = consts.tile([128, NCONST], BF16, tag="kc")
            nc.scalar.dma_start(kc_sb[:], consts_d.ap()[:, :])
            pe_sb = consts.tile([128, ELEM], F32, tag="pe")
            nc.scalar.dma_start(pe_sb[:], pe258_d.ap()[:, :])
            smats_sb = consts.tile([128, 12, 128], BF16, tag="smats")
            nc.sync.dma_start(
                smats_sb[:],
                bass.AP(tensor=smats_d, offset=0,
                        ap=[[128, 128], [128 * 128, 12], [1, 128]]))

            kc = kc_sb[:]
            taug = _ap2(kc, O_TAUG, [[ELEM, 2], [1, ELEM]])
            iota = _ap2(kc, O_IOTA, [[128, 2], [1, 128]])
            m2b = _ap2(kc, O_M2B, [[4, 64], [1, 4]])
            oh4 = kc_sb[0:4, O_OH4:O_OH4 + 128]
            peaug = kc_sb[0:4, O_PEAUG:O_PEAUG + ELEM]

            # ---- persistent big tensors ----
            X = big.tile([128, NCHUNK, ELEM], BF16, tag="X")
            C = big.tile([128, NCHUNK, 64, 4], BF16, tag="C")
            Gsb = big.tile([128, NCHUNK, 128], BF16, tag="Gsb")
            c4 = big.tile([128, NCHUNK, 4], BF16, tag="c4")   # slot-ordered
            oh_tiles = {}

            def mmat(out_ap, mi, rhs_ap, start, stop):
                nc.tensor.matmul(out=out_ap, lhsT=smats_sb[:, mi, :],
                                 rhs=rhs_ap, start=start, stop=stop,
                                 skip_group_check=True)

            def emit_onehots(g):
                g0 = g * GSZ
                for cc in range(0, GSZ, 2):
                    # one-hot for 2 chunks per DVE op
                    oht = oh_pool.tile([128, 2, 2, 128], BF16, tag="oh")
                    oh_tiles[g0 + cc] = oht
                    nc.vector.tensor_tensor(
                        out=oht[:],
                        in0=idxr_sb[:, g0 + cc:g0 + cc + 2, :].unsqueeze(
                            1).to_broadcast([128, 2, 2, 128]),
                        in1=iota.unsqueeze(2).to_broadcast([128, 2, 2, 128]),
                        op=mybir.AluOpType.is_equal)

            def emit_embed(g):
                g0 = g * GSZ
                for c in range(GSZ):
                    ch = g0 + c
                    oht = oh_tiles[ch - ch % 2]
                    k = ch % 2
                    dve_copy = (ch % DVE_XCOPY == DVE_XCOPY - 1)
                    ps = emb_ps.tile([128, ELEM], F32, tag="emb",
                                     padded_shape=[128, 512])
                    if not dve_copy:
                        # pe via extra K=4 matmul; plain ACT copy after
                        nc.tensor.matmul(out=ps[:], lhsT=oh4, rhs=peaug,
                                         start=True, stop=False,
                                         skip_group_check=True)
                    nc.tensor.matmul(out=ps[:], lhsT=oht[:, 0, k, :],
                                     rhs=_ap2(taug, 0, [[1, ELEM]]),
                                     start=dve_copy, stop=False,
                                     skip_group_check=True)
                    nc.tensor.matmul(out=ps[:], lhsT=oht[:, 1, k, :],
                                     rhs=_ap2(taug, ELEM, [[1, ELEM]]),
                                     start=False, stop=True,
                                     skip_group_check=True)
                    if dve_copy:
                        # DVE: X = psum + pe, cast bf16 (pe-add fused free)
                        nc.vector.tensor_tensor(
                            out=X[:, ch, :], in0=ps[:], in1=pe_sb[:],
                            op=mybir.AluOpType.add)
                    else:
                        nc.scalar.copy(X[:, ch, :], ps[:])

            def emit_scores(ga):
                """Scores for the self-contained group pair (ga, ga+1):
                rhs APs read X's score column (=2z) directly."""
                a0 = ga * GSZ
                W = 2 * GSZ  # 24
                # scT [128, 4, W] carved out of the gm bank (disjoint
                # lifetimes; Tile serializes the aliased uses)
                scT_full = gmid_ps.tile([128, GSZ, 64], F32, tag="gm")
                sc_b = scT_full[:]
                x_b = X[:]

                def sc_ap(m, o0, on):
                    return _ap2(sc_b, m * W + o0, [[GSZ, 2], [1, on]])

                def x_ap(t0, on):
                    # chunks a0+t0, +3, ... for both groups of the pair
                    return _ap2(x_b, (a0 + t0) * ELEM + EMBED,
                                [[GSZ * ELEM, 2], [3 * ELEM, on]])

                first = True
                for m, mi in ((0, 0), (1, 1), (3, 2)):
                    for c in range(3):
                        t0 = CLASS_TAUL[c][0]
                        mmat(sc_ap(m, 4 * c, 4), mi, x_ap(t0, 4), first,
                             False)
                        first = False
                for c in range(3):
                    t0 = CLASS_TAUL[c][0]
                    phi = phi_of_taul(t0)
                    mmat(sc_ap(2, 4 * c, 4), 3 + phi, x_ap(t0, 4),
                         False, False)
                up_sc = [(0, 0, 4, 8), (1, 4, 3, 1), (2, 8, 4, 4)]
                dn_sc = [(0, 1, 3, 4), (1, 4, 4, 8), (2, 8, 4, 0)]
                for plan, base in ((up_sc, 6), (dn_sc, 9)):
                    for c, o0, on, s0 in plan:
                        phi = phi_of_taul(CLASS_TAUL[c][0])
                        t0 = CLASS_TAUL[s0 // 4][s0 % 4]
                        mmat(sc_ap(2, o0, on), base + phi, x_ap(t0, on),
                             False, False)
                # pair-internal boundary fixes: ga is even (up-fix slot 7 <-
                # gb chunk 0 = tau 0), gb odd (dn-fix slot 0 <- ga chunk 11)
                mmat(_ap2(sc_b, 2 * W + 7, [[1, 1]]), 6 + phi_of_taul(11),
                     _ap2(x_b, (a0 + GSZ) * ELEM + EMBED, [[1, 1]]),
                     False, False)
                mmat(_ap2(sc_b, 2 * W + GSZ, [[1, 1]]), 9 + phi_of_taul(0),
                     _ap2(x_b, (a0 + 11) * ELEM + EMBED, [[1, 1]]),
                     False, True)

                # softmax + calibration (scS transposed to [128, slot, m])
                scS = sm.tile([128, W, 4], F32, tag="scS")
                scT_t = _ap2(sc_b, 0, [[1, W], [W, 4]])
                nc.vector.tensor_copy(out=scS[:], in_=scT_t)
                ex = sm.tile([128, W, 4], F32, tag="ex")
                nc.scalar.activation(out=ex[:], in_=scS[:],
                                     func=mybir.ActivationFunctionType.Exp)
                Z = sm.tile([128, W], F32, tag="Z")
                nc.vector.tensor_reduce(out=Z[:], in_=ex[:],
                                        axis=mybir.AxisListType.X,
                                        op=mybir.AluOpType.add)
                rz = sm.tile([128, W], F32, tag="rz")
                nc.vector.reciprocal(out=rz[:], in_=Z[:])
                r = sm.tile([128, W, 4], F32, tag="r")
                nc.vector.tensor_tensor(
                    out=r[:], in0=ex[:],
                    in1=rz[:].unsqueeze(2).to_broadcast([128, W, 4]),
                    op=mybir.AluOpType.mult)
                P = sm.tile([128, W, 4, 4], F32, tag="P")
                nc.vector.tensor_tensor(
                    out=P[:],
                    in0=r[:].unsqueeze(3).to_broadcast([128, W, 4, 4]),
                    in1=r[:].unsqueeze(2).to_broadcast([128, W, 4, 4]),
                    op=mybir.AluOpType.mult)
                E = sm.tile([128, W, 4, 4], F32, tag="E")
                nc.scalar.activation(out=E[:], in_=P[:],
                                     func=mybir.ActivationFunctionType.Exp)
                D = sm.tile([128, W, 4], F32, tag="D")
                nc.vector.tensor_reduce(out=D[:], in_=E[:],
                                        axis=mybir.AxisListType.X,
                                        op=mybir.AluOpType.add)
                EN = sm.tile([128, W, 4, 4], F32, tag="EN")
                nc.vector.tensor_tensor(
                    out=EN[:], in0=E[:],
                    in1=r[:].unsqueeze(2).to_broadcast([128, W, 4, 4]),
                    op=mybir.AluOpType.mult)
                Nn = sm.tile([128, W, 4], F32, tag="Nn")
                nc.vector.tensor_reduce(out=Nn[:], in_=EN[:],
                                        axis=mybir.AxisListType.X,
                                        op=mybir.AluOpType.add)
                rD = sm.tile([128, W, 4], F32, tag="rD")
                nc.vector.reciprocal(out=rD[:], in_=D[:])
                nc.vector.tensor_tensor(out=c4[:, a0:a0 + W, :], in0=Nn[:],
                                        in1=rD[:], op=mybir.AluOpType.mult)

                # C[l, j, m] = c4[l, m] * m2[l, j] on the Pool engine (all
                # SBUF); one op per group so builds(ga) starts before gb's C
                for gg in range(2):
                    b0 = a0 + gg * GSZ
                    nc.gpsimd.tensor_tensor(
                        out=C[:, b0:b0 + GSZ, :, :],
                        in0=c4[:, b0:b0 + GSZ, :].unsqueeze(2).to_broadcast(
                            [128, GSZ, 64, 4]),
                        in1=m2b.unsqueeze(1).to_broadcast(
                            [128, GSZ, 64, 4]),
                        op=mybir.AluOpType.mult)

            def emit_builds(g):
                g0 = g * GSZ
                gm = gmid_ps.tile([128, GSZ, 64], F32, tag="gm")
                # m2 / m4 over slots 0-7 then 8-11 (bank split)
                mmat(gm[:, 0:8, :], 1, C[:, g0:g0 + 8, :, 1], True, False)
                mmat(gm[:, 8:12, :], 1, C[:, g0 + 8:g0 + 12, :, 1], True, False)
                mmat(gm[:, 0:8, :], 2, C[:, g0:g0 + 8, :, 3], False, False)
                mmat(gm[:, 8:12, :], 2, C[:, g0 + 8:g0 + 12, :, 3], False, False)
                for c in range(3):
                    phi = phi_of_taul(CLASS_TAUL[c][0])
                    mmat(gm[:, 4 * c:4 * c + 4, :], 3 + phi,
                         C[:, g0 + 4 * c:g0 + 4 * c + 4, :, 2], False, True)
                gl = glow_ps.tile([128, GSZ, 32], F32, tag="gl")
                dn_plan = [(0, 1, 3, 4), (1, 4, 4, 8), (2, 8, 4, 0)]
                for i, (c, o0, on, s0) in enumerate(dn_plan):
                    phi = phi_of_taul(CLASS_TAUL[c][0])
                    mmat(gl[:, o0:o0 + on, :], 9 + phi,
                         C[:, g0 + s0:g0 + s0 + on, 32:64, 2], i == 0, True)
                if g % 2 == 1:
                    mmat(gl[:, 0:1, :], 9 + phi_of_taul(0),
                         C[:, g0 - 12 + 7:g0 - 12 + 8, 32:64, 2], False, True)
                gh = ghigh_ps.tile([128, GSZ, 32], F32, tag="gh")
                up_plan = [(0, 0, 4, 8), (1, 4, 3, 1), (2, 8, 4, 4)]
                for i, (c, o0, on, s0) in enumerate(up_plan):
                    phi = phi_of_taul(CLASS_TAUL[c][0])
                    mmat(gh[:, o0:o0 + on, :], 6 + phi,
                         C[:, g0 + s0:g0 + s0 + on, 0:32, 2], i == 0, True)
                if g % 2 == 0:
                    mmat(gh[:, 7:8, :], 6 + phi_of_taul(11),
                         C[:, g0 + 12:g0 + 13, 0:32, 2], False, True)
                    nc.vector.memset(gl[:, 0, :], 0.0)
                else:
                    nc.vector.memset(gh[:, 7, :], 0.0)
                # assemble Gsb (bf16)
                nc.vector.scalar_tensor_tensor(
                    out=Gsb[:, g0:g0 + GSZ, 32:96],
                    in0=C[:, g0:g0 + GSZ, :, 0], scalar=0.5, in1=gm[:],
                    op0=mybir.AluOpType.mult, op1=mybir.AluOpType.add)
                nc.scalar.copy(Gsb[:, g0:g0 + GSZ, 0:32], gl[:])
                nc.scalar.copy(Gsb[:, g0:g0 + GSZ, 96:128], gh[:])

            def gsb_idx(row, tt):
                g = 2 * row + tt // GSZ
                return g * GSZ + SLOT_OF[tt % GSZ]

            eng_ctr = [0]

            def emit_one_big(row, ot, out2):
                tt_e = 2 * ot
                if tt_e < 24:
                    nc.tensor.matmul(
                        out=out2[:, :],
                        lhsT=Gsb[:, gsb_idx(row, tt_e), :],
                        rhs=X[:, 24 * row + tt_e, 0:EMBED],
                        start=True, stop=False, skip_group_check=True)
                if tt_e - 1 >= 0:
                    nc.tensor.matmul(
                        out=out2[0:64, :],
                        lhsT=Gsb[:, gsb_idx(row, tt_e - 1), 64:128],
                        rhs=X[:, 24 * row + tt_e - 1, 0:EMBED],
                        start=(tt_e >= 24), stop=True,
                        skip_group_check=True)
                if tt_e + 1 < 24:
                    nc.tensor.matmul(
                        out=out2[64:128, :],
                        lhsT=Gsb[:, gsb_idx(row, tt_e + 1), 0:64],
                        rhs=X[:, 24 * row + tt_e + 1, 0:EMBED],
                        start=False, stop=True, skip_group_check=True)

            def copy_osb(dst, src):
                if eng_ctr[0] % 2 == 0:
                    nc.vector.tensor_copy(out=dst, in_=src)
                else:
                    nc.scalar.copy(dst, src)
                eng_ctr[0] += 1

            def emit_big(row, ot_list):
                # pair full interior ots -> one DMA per 256-row pair
                i = 0
                while i < len(ot_list):
                    ot = ot_list[i]
                    full = 0 < ot < 12
                    if full and i + 1 < len(ot_list) and 0 < ot_list[i + 1] < 12:
                        osb = outsb_pool.tile([128, 2, EMBED], F32, tag="osb")
                        for k in (0, 1):
                            out2 = out2_ps.tile([128, EMBED], F32, tag="out2")
                            emit_one_big(row, ot_list[i + k], out2)
                            copy_osb(osb[:, k, :], out2[:])
                        base = row * (SEQ // 2) + 128 * ot - 32
                        nc.sync.dma_start(
                            bass.AP(tensor=out_d, offset=base * EMBED,
                                    ap=[[EMBED, 128], [128 * EMBED, 2],
                                        [1, EMBED]]),
                            osb[:])
                        i += 2
                    else:
                        osb = outsb_pool.tile([128, 2, EMBED], F32, tag="osb")
                        out2 = out2_ps.tile([128, EMBED], F32, tag="out2")
                        emit_one_big(row, ot, out2)
                        base = row * (SEQ // 2) + 128 * ot - 32
                        p0, p1 = (32, 128) if ot == 0 else (0, 32) \
                            if ot == 12 else (0, 128)
                        # copies start at partition 0 (engine partition-base
                        # rule); ot==0 copies garbage rows 0:32, DMA skips
                        c0, c1 = (0, 32) if ot == 12 else (0, 128)
                        copy_osb(osb[c0:c1, 0, :], out2[c0:c1, :])
                        nc.sync.dma_start(out_d.ap()[base + p0:base + p1, :],
                                          osb[p0:p1, 0, :])
                        i += 1

            # ---- staged pipeline ----
            for _rep in range(nrep):
                emit_onehots(0)
                emit_onehots(1)
                emit_embed(0)
                emit_embed(1)
                emit_onehots(2)
                emit_scores(0)
                emit_embed(2)
                emit_onehots(3)
                emit_embed(3)
                emit_scores(2)
                emit_builds(0)
                emit_big(0, list(range(0, 6)))
                emit_builds(1)
                emit_big(0, list(range(6, 13)))
                emit_builds(2)
                emit_big(1, list(range(0, 6)))
                emit_builds(3)
                emit_big(1, list(range(6, 13)))

    return nc


_CACHE = {}


def _get_nc(nrep=1):
    key = f"nc{nrep}"
    if key not in _CACHE:
        nc = bacc.Bacc("TRN2", target_bir_lowering=False, debug=False)
        emit_program(nc, nrep=nrep)
        nc.compile()
        _CACHE[key] = nc
    return _CACHE[key]


def prepare_in_maps(input_ids, embed_table, w_score, b_score=None):
    # b_score only shifts all 4 scores equally -> softmax-invariant; unused.
    kconsts = build_consts(embed_table, w_score)
    pe258 = build_pe258(w_score)
    smats = build_smats()
    idx_streams = build_idx_streams(input_ids)
    return [{"idxs": idx_streams[core], "kconsts": kconsts, "pe258": pe258,
             "smats": smats} for core in range(NCORES)]


def assemble_out(results):
    outs = [results[c]["out"].reshape(BLOC, SEQ // 2, EMBED)
            for c in range(NCORES)]
    return np.concatenate(outs, axis=0)


def kernel(input_ids, embed_table, w_score, b_score):
    in_maps = prepare_in_maps(input_ids, embed_table, w_score, b_score)
    res = run_bass_kernel_spmd(_get_nc(), in_maps,
                               core_ids=list(range(NCORES)))
    return assemble_out(res.results)
